# revision 24
# baseline (speedup 1.0000x reference)
"""Trainium2 kernel for retrieval_knn (nn_GSL_7060926234912).

adj = W @ W.T  (W: [16384, 128] f32); per-row top-32 -> edge_index [2, N*K],
edge_values [N*K], columns sorted ascending within each row.

Sharding: rows of the similarity matrix split across 8 NeuronCores
(2048 rows/core). Each core gets the full W (the "database" side) plus its
row shard, computes its [2048, 16384] similarity block tile-by-tile on the
tensor engine (fp32 into PSUM), and runs per-row top-k selection on the
vector engine:
  stage 1: per 512-column chunk, top-8 values (nc.vector.max) + their
           chunk-local positions (nc.vector.max_index), straight from PSUM.
           -> 256 candidates/row. (P(some chunk holds >8 of a row's true
           top-32) ~ 1e-6/row — verified exact against the reference.)
  stage 2: top-32 of the 256 candidates via 4 rounds of
           max / max_index / match_replace.
Host side only reassembles: candidate position -> global column index
(pure integer arithmetic), per-row ordering by column index, and shard
concatenation.
"""

import numpy as np

import concourse.bass as bass
import concourse.mybir as mybir
from concourse import bacc
from concourse.masks import make_identity
from concourse.tile import TileContext
from concourse.bass_utils import run_bass_kernel_spmd

N = 16384
D = 128
K = 32
NCORES = 8
RPC = N // NCORES          # rows per core
RT = RPC // 128            # row tiles per core
CHW = 512                  # chunk width (columns per matmul / PSUM bank)
NCH = N // CHW             # chunks per row
NCAND = NCH * 8            # candidates per row (256)
NEG = -1.0e30
# Similarity matmul mode:
#   "f32"   - plain fp32 matmuls, 4 cy/row (exact)
#   "f16x3" - split W = hi + lo (fp16 each); adj = hi.hi + hi.lo + lo.hi,
#             three 1 cy/row matmuls accumulating in fp32 PSUM. The dropped
#             lo.lo term is ~1e-6 abs vs top-32 value gaps of ~0.1.
MM_MODE = "f16x3"
# Stage the matmul output through SBUF (ScalarE copy) before the DVE scans,
# instead of scanning PSUM directly: cheaper DVE access + PSUM banks recycle
# faster (PE less bank-starved).
SBUF_STAGE = False

_PROGRAM = None
_LAST = {"exec_time_ns": None, "profile_json": None}


def _ensure_ntff_hook():
    """The agent image's ``antenv`` lacks ``axon_hooks``, so trn_boot's NTFF
    registration silently degraded. Recreate the module and register the
    ctypes hook against the injected libaxon_pjrt.so so trace=True works."""
    import sys
    import types

    if "antenv.axon_hooks" in sys.modules:
        return
    mod = types.ModuleType("antenv.axon_hooks")
    mod._hook = None

    def set_axon_ntff_profile_hook(h):
        mod._hook = h

    def get_axon_ntff_profile_hook():
        return mod._hook

    mod.set_axon_ntff_profile_hook = set_axon_ntff_profile_hook
    mod.get_axon_ntff_profile_hook = get_axon_ntff_profile_hook
    sys.modules["antenv.axon_hooks"] = mod
    import antenv

    antenv.axon_hooks = mod
    sys.path.insert(0, "/root/.axon_site")
    from trn_agent_boot.trn_boot import _ntff_profile_via_ctypes

    hook = _ntff_profile_via_ctypes("/opt/axon/libaxon_pjrt.so")
    if hook is not None:
        mod._hook = hook


def build_program():
    nc = bacc.Bacc("TRN2", target_bir_lowering=False, debug=False,
                   num_devices=NCORES)

    W = nc.dram_tensor("W", [N, D], mybir.dt.float32, kind="ExternalInput").ap()
    Wr = nc.dram_tensor("Wr", [RPC, D], mybir.dt.float32,
                        kind="ExternalInput").ap()
    out_vals = nc.dram_tensor("out_vals", [RPC, K], mybir.dt.float32,
                              kind="ExternalOutput").ap()
    out_pos = nc.dram_tensor("out_pos", [RPC, K], mybir.dt.uint32,
                             kind="ExternalOutput").ap()
    out_cidx = nc.dram_tensor("out_cidx", [RPC, NCAND], mybir.dt.uint32,
                              kind="ExternalOutput").ap()

    with TileContext(nc) as tc:
        with (
            tc.tile_pool(name="const", bufs=1) as constp,
            tc.tile_pool(name="wt", bufs=1) as wtp,
            tc.tile_pool(name="wload", bufs=4) as loadp,
            tc.tile_pool(name="cand", bufs=6) as candp,
            tc.tile_pool(name="chunks", bufs=8) as chunkp,
            tc.tile_pool(name="mmps", bufs=6, space="PSUM") as mmps,
            tc.tile_pool(name="tpps", bufs=2, space="PSUM") as tpps,
        ):
            ident = constp.tile([128, 128], mybir.dt.float32)
            make_identity(nc, ident)

            # Full W, transposed: WT[d, j] = W[j, d]; likewise the row shard.
            if MM_MODE == "f16x3":
                WTh = wtp.tile([128, N], mybir.dt.float16)
                WTl = wtp.tile([128, N], mybir.dt.float16)
                WrTh = wtp.tile([128, RPC], mybir.dt.float16)
                WrTl = wtp.tile([128, RPC], mybir.dt.float16)

                def load_transposed(src, dsts, ntiles):
                    dh, dl = dsts
                    for t in range(ntiles):
                        cs = slice(t * 128, (t + 1) * 128)
                        wtile = loadp.tile([128, 128], mybir.dt.float32,
                                           tag="wload")
                        nc.sync.dma_start(out=wtile, in_=src[cs, :])
                        pt = tpps.tile([128, 128], mybir.dt.float32, tag="tp")
                        nc.tensor.transpose(pt, wtile, ident)
                        sc = loadp.tile([128, 128], mybir.dt.float32, tag="sc")
                        nc.scalar.copy(out=sc, in_=pt)
                        nc.scalar.copy(out=dh[:, cs], in_=sc)  # round to fp16
                        nc.gpsimd.tensor_sub(out=dl[:, cs], in0=sc,
                                             in1=dh[:, cs])

                load_transposed(Wr, (WrTh, WrTl), RPC // 128)
                load_transposed(W, (WTh, WTl), N // 128)
            else:
                WT = wtp.tile([128, N], mybir.dt.float32)
                WrT = wtp.tile([128, RPC], mybir.dt.float32)

                def load_transposed(src, dst, ntiles):
                    for t in range(ntiles):
                        cs = slice(t * 128, (t + 1) * 128)
                        wtile = loadp.tile([128, 128], mybir.dt.float32,
                                           tag="wload")
                        nc.sync.dma_start(out=wtile, in_=src[cs, :])
                        pt = tpps.tile([128, 128], mybir.dt.float32, tag="tp")
                        nc.tensor.transpose(pt, wtile, ident)
                        nc.scalar.copy(out=dst[:, cs], in_=pt)

                load_transposed(Wr, WrT, RPC // 128)
                load_transposed(W, WT, N // 128)

            for rt in range(RT):
                rcs = slice(rt * 128, (rt + 1) * 128)
                cv = candp.tile([128, NCAND], mybir.dt.float32, tag="cv")
                cl = candp.tile([128, NCAND], mybir.dt.uint32, tag="cl")
                for ch in range(NCH):
                    ps = mmps.tile([128, CHW], mybir.dt.float32, tag="mm")
                    ccs = slice(ch * CHW, (ch + 1) * CHW)
                    if MM_MODE == "f16x3":
                        nc.tensor.matmul(ps, WrTh[:, rcs], WTh[:, ccs],
                                         start=True, stop=False)
                        nc.tensor.matmul(ps, WrTh[:, rcs], WTl[:, ccs],
                                         start=False, stop=False)
                        nc.tensor.matmul(ps, WrTl[:, rcs], WTh[:, ccs],
                                         start=False, stop=True)
                    else:
                        nc.tensor.matmul(ps, WrT[:, rcs], WT[:, ccs],
                                         start=True, stop=True)
                    s8 = slice(ch * 8, (ch + 1) * 8)
                    if SBUF_STAGE:
                        sb = chunkp.tile([128, CHW], mybir.dt.float32,
                                         tag="chunk")
                        nc.scalar.copy(out=sb, in_=ps)
                        src = sb
                    else:
                        src = ps
                    nc.vector.max(out=cv[:, s8], in_=src)
                    nc.vector.max_index(out=cl[:, s8], in_max=cv[:, s8],
                                        in_values=src)

                cur = candp.tile([128, NCAND], mybir.dt.float32, tag="cur")
                nc.vector.tensor_copy(cur, cv)
                w32 = candp.tile([128, K], mybir.dt.float32, tag="w32")
                p32 = candp.tile([128, K], mybir.dt.uint32, tag="p32")
                for r in range(K // 8):
                    s8 = slice(r * 8, (r + 1) * 8)
                    nc.vector.max(out=w32[:, s8], in_=cur)
                    nc.vector.max_index(out=p32[:, s8], in_max=w32[:, s8],
                                        in_values=cur)
                    if r < K // 8 - 1:
                        nc.vector.match_replace(out=cur, in_to_replace=w32[:, s8],
                                                in_values=cur, imm_value=NEG)

                rs = slice(rt * 128, (rt + 1) * 128)
                nc.sync.dma_start(out=out_vals[rs, :], in_=w32)
                nc.sync.dma_start(out=out_pos[rs, :], in_=p32)
                nc.sync.dma_start(out=out_cidx[rs, :], in_=cl)

    nc.compile()
    return nc


def _get_program():
    global _PROGRAM
    if _PROGRAM is None:
        _PROGRAM = build_program()
    return _PROGRAM


def kernel(x=None, W=None, trace=False):
    W = np.ascontiguousarray(np.asarray(W, dtype=np.float32))
    assert W.shape == (N, D)
    if trace:
        _ensure_ntff_hook()
    nc = _get_program()

    in_maps = [
        {"W": W, "Wr": W[c * RPC:(c + 1) * RPC]} for c in range(NCORES)
    ]
    res = run_bass_kernel_spmd(nc, in_maps, list(range(NCORES)), trace=trace)
    _LAST["exec_time_ns"] = res.exec_time_ns
    _LAST["profile_json"] = res.profile_json
    _LAST["instructions"] = res.instructions_and_trace

    vals = np.concatenate([r["out_vals"] for r in res.results])   # [N, K] f32
    pos = np.concatenate([r["out_pos"] for r in res.results])     # [N, K] u32
    cidx = np.concatenate([r["out_cidx"] for r in res.results])   # [N, 256] u32

    # candidate position -> global column index
    local = np.take_along_axis(cidx, pos, axis=1).astype(np.int64)
    gidx = (pos.astype(np.int64) // 8) * CHW + local              # [N, K]
    order = np.argsort(gidx, axis=1)
    idx_s = np.take_along_axis(gidx, order, axis=1).astype(np.int32)
    vals_s = np.take_along_axis(vals, order, axis=1)

    rows = np.repeat(np.arange(N, dtype=np.int32), K)
    edge_index = np.stack([rows, idx_s.reshape(-1)])
    edge_values = vals_s.reshape(-1).astype(np.float32)
    return edge_index, edge_values


# revision 25
# speedup vs baseline: 1.0304x; 1.0304x over previous
"""Trainium2 kernel for retrieval_knn (nn_GSL_7060926234912).

adj = W @ W.T  (W: [16384, 128] f32); per-row top-32 -> edge_index [2, N*K],
edge_values [N*K], columns sorted ascending within each row.

Sharding: rows of the similarity matrix split across 8 NeuronCores
(2048 rows/core). Each core gets the full W (the "database" side) plus its
row shard, computes its [2048, 16384] similarity block tile-by-tile on the
tensor engine (fp32 into PSUM), and runs per-row top-k selection on the
vector engine:
  stage 1: per 512-column chunk, top-8 values (nc.vector.max) + their
           chunk-local positions (nc.vector.max_index), straight from PSUM.
           -> 256 candidates/row. (P(some chunk holds >8 of a row's true
           top-32) ~ 1e-6/row — verified exact against the reference.)
  stage 2: top-32 of the 256 candidates via 4 rounds of
           max / max_index / match_replace.
Host side only reassembles: candidate position -> global column index
(pure integer arithmetic), per-row ordering by column index, and shard
concatenation.
"""

import numpy as np

import concourse.bass as bass
import concourse.mybir as mybir
from concourse import bacc
from concourse.masks import make_identity
from concourse.tile import TileContext
from concourse.bass_utils import run_bass_kernel_spmd

N = 16384
D = 128
K = 32
NCORES = 8
RPC = N // NCORES          # rows per core
RT = RPC // 128            # row tiles per core
CHW = 512                  # chunk width (columns per matmul / PSUM bank)
NCH = N // CHW             # chunks per row
NCAND = NCH * 8            # candidates per row (256)
NEG = -1.0e30
# Similarity matmul mode:
#   "f32"   - plain fp32 matmuls, 4 cy/row (exact)
#   "f16x3" - split W = hi + lo (fp16 each); adj = hi.hi + hi.lo + lo.hi,
#             three 1 cy/row matmuls accumulating in fp32 PSUM. The dropped
#             lo.lo term is ~1e-6 abs vs top-32 value gaps of ~0.1.
MM_MODE = "f16x3"
# Stage the matmul output through SBUF (ScalarE copy) before the DVE scans,
# instead of scanning PSUM directly: cheaper DVE access + PSUM banks recycle
# faster (PE less bank-starved).
SBUF_STAGE = False

_PROGRAM = None
_LAST = {"exec_time_ns": None, "profile_json": None}


def _ensure_ntff_hook():
    """The agent image's ``antenv`` lacks ``axon_hooks``, so trn_boot's NTFF
    registration silently degraded. Recreate the module and register the
    ctypes hook against the injected libaxon_pjrt.so so trace=True works."""
    import sys
    import types

    if "antenv.axon_hooks" in sys.modules:
        return
    mod = types.ModuleType("antenv.axon_hooks")
    mod._hook = None

    def set_axon_ntff_profile_hook(h):
        mod._hook = h

    def get_axon_ntff_profile_hook():
        return mod._hook

    mod.set_axon_ntff_profile_hook = set_axon_ntff_profile_hook
    mod.get_axon_ntff_profile_hook = get_axon_ntff_profile_hook
    sys.modules["antenv.axon_hooks"] = mod
    import antenv

    antenv.axon_hooks = mod
    sys.path.insert(0, "/root/.axon_site")
    from trn_agent_boot.trn_boot import _ntff_profile_via_ctypes

    hook = _ntff_profile_via_ctypes("/opt/axon/libaxon_pjrt.so")
    if hook is not None:
        mod._hook = hook


def build_program():
    nc = bacc.Bacc("TRN2", target_bir_lowering=False, debug=False,
                   num_devices=NCORES)

    W = nc.dram_tensor("W", [N, D], mybir.dt.float32, kind="ExternalInput").ap()
    Wr = nc.dram_tensor("Wr", [RPC, D], mybir.dt.float32,
                        kind="ExternalInput").ap()
    out_vals = nc.dram_tensor("out_vals", [RPC, K], mybir.dt.float32,
                              kind="ExternalOutput").ap()
    out_pos = nc.dram_tensor("out_pos", [RPC, K], mybir.dt.uint32,
                             kind="ExternalOutput").ap()
    out_cidx = nc.dram_tensor("out_cidx", [RPC, NCAND], mybir.dt.uint32,
                              kind="ExternalOutput").ap()

    with TileContext(nc) as tc:
        with (
            tc.tile_pool(name="const", bufs=1) as constp,
            tc.tile_pool(name="wt", bufs=1) as wtp,
            tc.tile_pool(name="wload", bufs=4) as loadp,
            tc.tile_pool(name="cand", bufs=4) as candp,
            tc.tile_pool(name="chunks", bufs=8) as chunkp,
            tc.tile_pool(name="mmps", bufs=6, space="PSUM") as mmps,
            tc.tile_pool(name="tpps", bufs=2, space="PSUM") as tpps,
        ):
            ident = constp.tile([128, 128], mybir.dt.float32)
            make_identity(nc, ident)

            # Full W, transposed: WT[d, j] = W[j, d]; likewise the row shard.
            if MM_MODE == "f16x3":
                WTh = wtp.tile([128, N], mybir.dt.float16)
                WTl = wtp.tile([128, N], mybir.dt.float16)
                WrTh = wtp.tile([128, RPC], mybir.dt.float16)
                WrTl = wtp.tile([128, RPC], mybir.dt.float16)

                def load_transposed(src, dsts, ntiles):
                    dh, dl = dsts
                    for t in range(ntiles):
                        cs = slice(t * 128, (t + 1) * 128)
                        wtile = loadp.tile([128, 128], mybir.dt.float32,
                                           tag="wload")
                        nc.sync.dma_start(out=wtile, in_=src[cs, :])
                        pt = tpps.tile([128, 128], mybir.dt.float32, tag="tp")
                        nc.tensor.transpose(pt, wtile, ident)
                        sc = loadp.tile([128, 128], mybir.dt.float32, tag="sc")
                        nc.scalar.copy(out=sc, in_=pt)
                        nc.scalar.copy(out=dh[:, cs], in_=sc)  # round to fp16
                        nc.gpsimd.tensor_sub(out=dl[:, cs], in0=sc,
                                             in1=dh[:, cs])

                load_transposed(Wr, (WrTh, WrTl), RPC // 128)
                load_transposed(W, (WTh, WTl), N // 128)
            else:
                WT = wtp.tile([128, N], mybir.dt.float32)
                WrT = wtp.tile([128, RPC], mybir.dt.float32)

                def load_transposed(src, dst, ntiles):
                    for t in range(ntiles):
                        cs = slice(t * 128, (t + 1) * 128)
                        wtile = loadp.tile([128, 128], mybir.dt.float32,
                                           tag="wload")
                        nc.sync.dma_start(out=wtile, in_=src[cs, :])
                        pt = tpps.tile([128, 128], mybir.dt.float32, tag="tp")
                        nc.tensor.transpose(pt, wtile, ident)
                        nc.scalar.copy(out=dst[:, cs], in_=pt)

                load_transposed(Wr, WrT, RPC // 128)
                load_transposed(W, WT, N // 128)

            for rt in range(RT):
                rcs = slice(rt * 128, (rt + 1) * 128)
                cv = candp.tile([128, NCAND], mybir.dt.float32, tag="cv")
                cl = candp.tile([128, NCAND], mybir.dt.uint32, tag="cl")
                for ch in range(NCH):
                    ps = mmps.tile([128, CHW], mybir.dt.float32, tag="mm")
                    ccs = slice(ch * CHW, (ch + 1) * CHW)
                    if MM_MODE == "f16x3":
                        nc.tensor.matmul(ps, WrTh[:, rcs], WTh[:, ccs],
                                         start=True, stop=False)
                        nc.tensor.matmul(ps, WrTh[:, rcs], WTl[:, ccs],
                                         start=False, stop=False)
                        nc.tensor.matmul(ps, WrTl[:, rcs], WTh[:, ccs],
                                         start=False, stop=True)
                    else:
                        nc.tensor.matmul(ps, WrT[:, rcs], WT[:, ccs],
                                         start=True, stop=True)
                    s8 = slice(ch * 8, (ch + 1) * 8)
                    if SBUF_STAGE:
                        sb = chunkp.tile([128, CHW], mybir.dt.float32,
                                         tag="chunk")
                        nc.scalar.copy(out=sb, in_=ps)
                        src = sb
                    else:
                        src = ps
                    nc.vector.max(out=cv[:, s8], in_=src)
                    nc.vector.max_index(out=cl[:, s8], in_max=cv[:, s8],
                                        in_values=src)

                cur = candp.tile([128, NCAND], mybir.dt.float32, tag="cur")
                nc.gpsimd.tensor_copy(cur, cv)
                w32 = candp.tile([128, K], mybir.dt.float32, tag="w32")
                p32 = candp.tile([128, K], mybir.dt.uint32, tag="p32")
                for r in range(K // 8):
                    s8 = slice(r * 8, (r + 1) * 8)
                    nc.vector.max(out=w32[:, s8], in_=cur)
                    nc.vector.max_index(out=p32[:, s8], in_max=w32[:, s8],
                                        in_values=cur)
                    if r < K // 8 - 1:
                        nc.vector.match_replace(out=cur, in_to_replace=w32[:, s8],
                                                in_values=cur, imm_value=NEG)

                rs = slice(rt * 128, (rt + 1) * 128)
                nc.sync.dma_start(out=out_vals[rs, :], in_=w32)
                nc.sync.dma_start(out=out_pos[rs, :], in_=p32)
                nc.sync.dma_start(out=out_cidx[rs, :], in_=cl)

    nc.compile()
    return nc


def _get_program():
    global _PROGRAM
    if _PROGRAM is None:
        _PROGRAM = build_program()
    return _PROGRAM


def kernel(x=None, W=None, trace=False):
    W = np.ascontiguousarray(np.asarray(W, dtype=np.float32))
    assert W.shape == (N, D)
    if trace:
        _ensure_ntff_hook()
    nc = _get_program()

    in_maps = [
        {"W": W, "Wr": W[c * RPC:(c + 1) * RPC]} for c in range(NCORES)
    ]
    res = run_bass_kernel_spmd(nc, in_maps, list(range(NCORES)), trace=trace)
    _LAST["exec_time_ns"] = res.exec_time_ns
    _LAST["profile_json"] = res.profile_json
    _LAST["instructions"] = res.instructions_and_trace

    vals = np.concatenate([r["out_vals"] for r in res.results])   # [N, K] f32
    pos = np.concatenate([r["out_pos"] for r in res.results])     # [N, K] u32
    cidx = np.concatenate([r["out_cidx"] for r in res.results])   # [N, 256] u32

    # candidate position -> global column index
    local = np.take_along_axis(cidx, pos, axis=1).astype(np.int64)
    gidx = (pos.astype(np.int64) // 8) * CHW + local              # [N, K]
    order = np.argsort(gidx, axis=1)
    idx_s = np.take_along_axis(gidx, order, axis=1).astype(np.int32)
    vals_s = np.take_along_axis(vals, order, axis=1)

    rows = np.repeat(np.arange(N, dtype=np.int32), K)
    edge_index = np.stack([rows, idx_s.reshape(-1)])
    edge_values = vals_s.reshape(-1).astype(np.float32)
    return edge_index, edge_values
